# revision 7
# baseline (speedup 1.0000x reference)
"""ChildSum tree RNN over a batch of complete binary trees — Trainium2 Bass kernel.

Strategy (data-parallel over trees, 8 cores x 128 trees):
  - States are kept *transposed* in SBUF ([feature, tree-node-column]) so every
    level's matmul chains directly into the next with no on-device transposes.
  - Host marshalling: per-core leaf states pre-transposed to [256, 16384], the
    two 256x256 weight matrices pre-transposed (and pre-halved copies for the
    sibling-mean folding), and the per-node op codes expanded into a
    partition-broadcast uint8 mask.
  - Per level: 8 matmuls (2 ops x 2 contraction chunks x 2 output chunks) into
    PSUM, a single copy_predicated selects the '|' result where op==1 (select
    commutes with the monotone tanh), one tanh (PSUM->SBUF), and one strided
    tensor_add implements the sibling mean (the x0.5 lives in the pre-halved
    weights of the next level / the host epilogue for the root).
"""

import sys

for _p in ("/opt/trn_rl_repo",):
    if _p not in sys.path:
        sys.path.insert(0, _p)

import numpy as np

import concourse.bacc as bacc
import concourse.mybir as mybir
import concourse.tile as tile
from concourse import bass_utils

N_CORES = 8
B, L, M = 1024, 128, 256
BC = B // N_CORES          # trees per core
R0 = BC * L                # level-0 child columns per core (16384)
DEPTH = 7
LEVEL_R = [R0 >> l for l in range(DEPTH)]      # child columns per level
LEVEL_N = [64 >> l for l in range(DEPTH)]      # parents per tree per level
LEVEL_OFF = [0, 64, 96, 112, 120, 124, 126]    # offsets into ops[:, :]
MTOT = BC * 254                                # child-res mask columns (32512)
BLK = 512                                      # child columns per block

MM_DT = mybir.dt.float32r  # full-rate PE; bit-identical storage to fp32


def _body(nc, xT, wa, wo, wa5, wo5, mk, outT, tc):
    f32 = mybir.dt.float32
    u8 = mybir.dt.uint8

    with (
        tc.tile_pool(name="wpool", bufs=1) as wpool,
        tc.tile_pool(name="spool", bufs=1) as spool,
        tc.tile_pool(name="xpool", bufs=4) as xpool,
        tc.tile_pool(name="vpool", bufs=3) as vpool,
        tc.tile_pool(name="mpool", bufs=3) as mpool,
        tc.tile_pool(name="ppool", bufs=2, space="PSUM") as ppool,
    ):
        # Stationary weights: [contract-chunk m (128 part), out-feature k (256)]
        wt = {}
        for nm, dram in (("wa", wa), ("wo", wo), ("wa5", wa5), ("wo5", wo5)):
            for C in range(2):
                t = wpool.tile([128, 256], MM_DT, name=f"{nm}{C}", tag=f"{nm}{C}")
                nc.sync.dma_start(out=t, in_=dram[C * 128:(C + 1) * 128, :])
                wt[(nm, C)] = t

        moff = 0
        s_prev = None
        for lvl in range(DEPTH):
            R = LEVEL_R[lvl]          # child columns this level
            P = R // 2                # parent columns this level
            wand = "wa" if lvl == 0 else "wa5"
            wor = "wo" if lvl == 0 else "wo5"
            s_tag = "se" if lvl % 2 == 0 else "so"
            s_cur = spool.tile([128, 2 * P], MM_DT, tag=s_tag, name=f"s{lvl}")

            nblk = (R + BLK - 1) // BLK
            for j in range(nblk):
                W = min(BLK, R - j * BLK)     # child cols in this block
                NP = W // 2                   # parent cols in this block

                # rhs tiles per contraction chunk
                if lvl == 0:
                    rhs = []
                    for C in range(2):
                        xt = xpool.tile([128, BLK], MM_DT, tag=f"x{C}",
                                        name=f"x{C}_{j}")
                        nc.sync.dma_start(
                            out=xt[:, 0:W],
                            in_=xT[C * 128:(C + 1) * 128, j * BLK:j * BLK + W])
                        rhs.append(xt[:, 0:W])
                else:
                    Rp = R  # s_prev has 2*R/2... s_prev half width == R
                    rhs = [s_prev[:, C * R + j * BLK: C * R + j * BLK + W]
                           for C in range(2)]

                ta = ppool.tile([128, 1024], f32, tag="ta", name=f"ta_{lvl}_{j}")
                to_ = ppool.tile([128, 1024], f32, tag="to", name=f"to_{lvl}_{j}")

                for psum_t, wnm in ((ta, wand), (to_, wor)):
                    for Mo in range(2):
                        out_ap = psum_t[:, Mo * W:(Mo + 1) * W]
                        for C in range(2):
                            nc.tensor.matmul(
                                out_ap,
                                wt[(wnm, C)][:, Mo * 128:(Mo + 1) * 128],
                                rhs[C],
                                start=(C == 0),
                                stop=(C == 1),
                            )

                # op mask for this block (child resolution, partition-
                # broadcast already done on the host)
                mt = mpool.tile([128, BLK], u8, tag="mk", name=f"mk_{lvl}_{j}")
                nc.sync.dma_start(
                    out=mt[:, 0:W],
                    in_=mk[:, moff + j * BLK: moff + j * BLK + W])

                # select '|' where op==1 (pre-tanh; tanh is monotone so the
                # select commutes), then tanh PSUM->SBUF
                for Mo in range(2):
                    nc.vector.copy_predicated(
                        ta[:, Mo * W:(Mo + 1) * W], mt[:, 0:W],
                        to_[:, Mo * W:(Mo + 1) * W])

                v = vpool.tile([128, 1024], MM_DT, tag="v", name=f"v_{lvl}_{j}")
                nc.scalar.activation(v[:, 0:2 * W], ta[:, 0:2 * W],
                                     mybir.ActivationFunctionType.Tanh)

                # sibling sum (the x0.5 of the mean is folded into the halved
                # weights of the next level / the host epilogue for the root)
                v4 = v[:, 0:2 * W].rearrange("p (h c t) -> p h c t", h=2, t=2)
                s_out = (s_cur.rearrange("p (h q) -> p h q", h=2)
                         [:, :, j * (BLK // 2): j * (BLK // 2) + NP])
                nc.vector.tensor_add(s_out, v4[:, :, :, 0], v4[:, :, :, 1])

            moff += R
            s_prev = s_cur

        nc.sync.dma_start(out=outT, in_=s_prev[:, 0:2 * BC])


_NC_CACHE = {}


def _get_nc():
    key = "nc"
    if key not in _NC_CACHE:
        f32 = mybir.dt.float32
        u8 = mybir.dt.uint8
        nc = bacc.Bacc("TRN2", target_bir_lowering=False, debug=False)
        xT = nc.dram_tensor("xT", [M, R0], MM_DT, kind="ExternalInput").ap()
        wa = nc.dram_tensor("wa", [M, M], MM_DT, kind="ExternalInput").ap()
        wo = nc.dram_tensor("wo", [M, M], MM_DT, kind="ExternalInput").ap()
        wa5 = nc.dram_tensor("wa5", [M, M], MM_DT, kind="ExternalInput").ap()
        wo5 = nc.dram_tensor("wo5", [M, M], MM_DT, kind="ExternalInput").ap()
        mk = nc.dram_tensor("mk", [128, MTOT], u8, kind="ExternalInput").ap()
        outT = nc.dram_tensor("outT", [128, 2 * BC], MM_DT,
                              kind="ExternalOutput").ap()
        with tile.TileContext(nc) as tc:
            _body(nc, xT, wa, wo, wa5, wo5, mk, outT, tc)
        nc.compile()
        _NC_CACHE[key] = nc
    return _NC_CACHE[key]


def make_in_maps(inputs, ops, W_and, W_or):
    x = np.asarray(inputs, dtype=np.float32)
    opsA = np.asarray(ops)
    waT = np.ascontiguousarray(np.asarray(W_and, dtype=np.float32).T)
    woT = np.ascontiguousarray(np.asarray(W_or, dtype=np.float32).T)
    wa5 = np.ascontiguousarray(0.5 * waT)
    wo5 = np.ascontiguousarray(0.5 * woT)
    in_maps = []
    for c in range(N_CORES):
        xc = np.ascontiguousarray(
            x[c * BC:(c + 1) * BC].reshape(BC * L, M).T)
        opc = opsA[c * BC:(c + 1) * BC]
        rows = []
        for lvl in range(DEPTH):
            n = LEVEL_N[lvl]
            off = LEVEL_OFF[lvl]
            child = np.repeat(opc[:, off:off + n].astype(np.uint8), 2, axis=1)
            rows.append(np.broadcast_to(
                child.reshape(1, BC * 2 * n), (128, BC * 2 * n)))
        mkc = np.ascontiguousarray(np.concatenate(rows, axis=1))
        in_maps.append({"xT": xc, "wa": waT, "wo": woT,
                        "wa5": wa5, "wo5": wo5, "mk": mkc})
    return in_maps


def postprocess(results):
    outs = []
    for c in range(N_CORES):
        r = np.asarray(results[c]["outT"]).reshape(128, 2, BC)
        outs.append(0.5 * np.transpose(r, (2, 1, 0)).reshape(BC, M))
    return np.concatenate(outs, axis=0).astype(np.float32)


def kernel(inputs, ops, W_and, W_or):
    nc = _get_nc()
    in_maps = make_in_maps(inputs, ops, W_and, W_or)
    res = bass_utils.run_bass_kernel_spmd(nc, in_maps, list(range(N_CORES)))
    return postprocess(res.results)


# revision 8
# speedup vs baseline: 69.9661x; 69.9661x over previous
"""ChildSum tree RNN over a batch of complete binary trees — Trainium2 Bass kernel.

Strategy (data-parallel over trees, 8 cores x 128 trees):
  - States are kept *transposed* in SBUF ([feature, tree-node-column]) so every
    level's matmul chains directly into the next with no on-device transposes.
  - Host marshalling: per-core leaf states pre-transposed to [256, 16384], the
    two 256x256 weight matrices pre-transposed (and pre-halved copies for the
    sibling-mean folding), and the per-node op codes expanded into a
    partition-broadcast uint8 mask.
  - Per level: 8 matmuls (2 ops x 2 contraction chunks x 2 output chunks) into
    PSUM, a single copy_predicated selects the '|' result where op==1 (select
    commutes with the monotone tanh), one tanh (PSUM->SBUF), and one strided
    tensor_add implements the sibling mean (the x0.5 lives in the pre-halved
    weights of the next level / the host epilogue for the root).
"""

import sys

for _p in ("/opt/trn_rl_repo",):
    if _p not in sys.path:
        sys.path.insert(0, _p)

import numpy as np

import concourse.bacc as bacc
import concourse.mybir as mybir
import concourse.tile as tile
from concourse import bass_utils

N_CORES = 8
B, L, M = 1024, 128, 256
BC = B // N_CORES          # trees per core
R0 = BC * L                # level-0 child columns per core (16384)
DEPTH = 7
LEVEL_R = [R0 >> l for l in range(DEPTH)]      # child columns per level
LEVEL_N = [64 >> l for l in range(DEPTH)]      # parents per tree per level
LEVEL_OFF = [0, 64, 96, 112, 120, 124, 126]    # offsets into ops[:, :]
MTOT = BC * 254                                # child-res mask columns (32512)
BLK = 512                                      # child columns per block

MM_DT = mybir.dt.float32r  # full-rate PE; bit-identical storage to fp32


def _body(nc, xT, wa, wo, wa5, wo5, mk, outT, tc):
    f32 = mybir.dt.float32
    u8 = mybir.dt.uint8

    with (
        tc.tile_pool(name="wpool", bufs=1) as wpool,
        tc.tile_pool(name="spool", bufs=1) as spool,
        tc.tile_pool(name="xpool", bufs=4) as xpool,
        tc.tile_pool(name="vpool", bufs=3) as vpool,
        tc.tile_pool(name="mpool", bufs=3) as mpool,
        tc.tile_pool(name="ppool", bufs=2, space="PSUM") as ppool,
    ):
        # Stationary weights: [contract-chunk m (128 part), out-feature k (256)]
        wt = {}
        for nm, dram in (("wa", wa), ("wo", wo), ("wa5", wa5), ("wo5", wo5)):
            for C in range(2):
                t = wpool.tile([128, 256], MM_DT, name=f"{nm}{C}", tag=f"{nm}{C}")
                nc.sync.dma_start(out=t, in_=dram[C * 128:(C + 1) * 128, :])
                wt[(nm, C)] = t

        moff = 0
        s_prev = None
        for lvl in range(DEPTH):
            R = LEVEL_R[lvl]          # child columns this level
            P = R // 2                # parent columns this level
            wand = "wa" if lvl == 0 else "wa5"
            wor = "wo" if lvl == 0 else "wo5"
            s_tag = "se" if lvl % 2 == 0 else "so"
            s_cur = spool.tile([128, 2 * P], MM_DT, tag=s_tag, name=f"s{lvl}")

            nblk = (R + BLK - 1) // BLK
            for j in range(nblk):
                W = min(BLK, R - j * BLK)     # child cols in this block
                NP = W // 2                   # parent cols in this block

                # rhs tiles per contraction chunk
                if lvl == 0:
                    rhs = []
                    for C in range(2):
                        xt = xpool.tile([128, BLK], MM_DT, tag=f"x{C}",
                                        name=f"x{C}_{j}")
                        nc.sync.dma_start(
                            out=xt[:, 0:W],
                            in_=xT[C * 128:(C + 1) * 128, j * BLK:j * BLK + W])
                        rhs.append(xt[:, 0:W])
                else:
                    Rp = R  # s_prev has 2*R/2... s_prev half width == R
                    rhs = [s_prev[:, C * R + j * BLK: C * R + j * BLK + W]
                           for C in range(2)]

                ta = ppool.tile([128, 1024], f32, tag="ta", name=f"ta_{lvl}_{j}")
                to_ = ppool.tile([128, 1024], f32, tag="to", name=f"to_{lvl}_{j}")

                for psum_t, wnm in ((ta, wand), (to_, wor)):
                    for Mo in range(2):
                        out_ap = psum_t[:, Mo * W:(Mo + 1) * W]
                        for C in range(2):
                            nc.tensor.matmul(
                                out_ap,
                                wt[(wnm, C)][:, Mo * 128:(Mo + 1) * 128],
                                rhs[C],
                                start=(C == 0),
                                stop=(C == 1),
                            )

                # op mask for this block (child resolution, partition-
                # broadcast already done on the host)
                mt = mpool.tile([128, BLK], u8, tag="mk", name=f"mk_{lvl}_{j}")
                nc.sync.dma_start(
                    out=mt[:, 0:W],
                    in_=mk[:, moff + j * BLK: moff + j * BLK + W])

                # select '|' where op==1 (pre-tanh; tanh is monotone so the
                # select commutes), then tanh PSUM->SBUF
                for Mo in range(2):
                    nc.vector.copy_predicated(
                        ta[:, Mo * W:(Mo + 1) * W], mt[:, 0:W],
                        to_[:, Mo * W:(Mo + 1) * W])

                v = vpool.tile([128, 1024], MM_DT, tag="v", name=f"v_{lvl}_{j}")
                nc.scalar.activation(v[:, 0:2 * W], ta[:, 0:2 * W],
                                     mybir.ActivationFunctionType.Tanh)

                # sibling sum (the x0.5 of the mean is folded into the halved
                # weights of the next level / the host epilogue for the root)
                v4 = v[:, 0:2 * W].rearrange("p (h c t) -> p h c t", h=2, t=2)
                s_out = (s_cur.rearrange("p (h q) -> p h q", h=2)
                         [:, :, j * (BLK // 2): j * (BLK // 2) + NP])
                nc.vector.tensor_add(s_out, v4[:, :, :, 0], v4[:, :, :, 1])

            moff += R
            s_prev = s_cur

        nc.sync.dma_start(out=outT, in_=s_prev[:, 0:2 * BC])


_NC_CACHE = {}


def _get_nc(reps=1):
    key = ("nc", reps)
    if key not in _NC_CACHE:
        f32 = mybir.dt.float32
        u8 = mybir.dt.uint8
        nc = bacc.Bacc("TRN2", target_bir_lowering=False, debug=False)
        xT = nc.dram_tensor("xT", [M, R0], MM_DT, kind="ExternalInput").ap()
        wa = nc.dram_tensor("wa", [M, M], MM_DT, kind="ExternalInput").ap()
        wo = nc.dram_tensor("wo", [M, M], MM_DT, kind="ExternalInput").ap()
        wa5 = nc.dram_tensor("wa5", [M, M], MM_DT, kind="ExternalInput").ap()
        wo5 = nc.dram_tensor("wo5", [M, M], MM_DT, kind="ExternalInput").ap()
        mk = nc.dram_tensor("mk", [128, MTOT], u8, kind="ExternalInput").ap()
        outT = nc.dram_tensor("outT", [128, 2 * BC], MM_DT,
                              kind="ExternalOutput").ap()
        with tile.TileContext(nc) as tc:
            for _ in range(reps):
                _body(nc, xT, wa, wo, wa5, wo5, mk, outT, tc)
        nc.compile()
        _NC_CACHE[key] = nc
    return _NC_CACHE[key]


def make_in_maps(inputs, ops, W_and, W_or):
    x = np.asarray(inputs, dtype=np.float32)
    opsA = np.asarray(ops)
    waT = np.ascontiguousarray(np.asarray(W_and, dtype=np.float32).T)
    woT = np.ascontiguousarray(np.asarray(W_or, dtype=np.float32).T)
    wa5 = np.ascontiguousarray(0.5 * waT)
    wo5 = np.ascontiguousarray(0.5 * woT)
    in_maps = []
    for c in range(N_CORES):
        xc = np.ascontiguousarray(
            x[c * BC:(c + 1) * BC].reshape(BC * L, M).T)
        opc = opsA[c * BC:(c + 1) * BC]
        rows = []
        for lvl in range(DEPTH):
            n = LEVEL_N[lvl]
            off = LEVEL_OFF[lvl]
            child = np.repeat(opc[:, off:off + n].astype(np.uint8), 2, axis=1)
            rows.append(np.broadcast_to(
                child.reshape(1, BC * 2 * n), (128, BC * 2 * n)))
        mkc = np.ascontiguousarray(np.concatenate(rows, axis=1))
        in_maps.append({"xT": xc, "wa": waT, "wo": woT,
                        "wa5": wa5, "wo5": wo5, "mk": mkc})
    return in_maps


def postprocess(results):
    outs = []
    for c in range(N_CORES):
        r = np.asarray(results[c]["outT"]).reshape(128, 2, BC)
        outs.append(0.5 * np.transpose(r, (2, 1, 0)).reshape(BC, M))
    return np.concatenate(outs, axis=0).astype(np.float32)


def kernel(inputs, ops, W_and, W_or):
    nc = _get_nc()
    in_maps = make_in_maps(inputs, ops, W_and, W_or)
    res = bass_utils.run_bass_kernel_spmd(nc, in_maps, list(range(N_CORES)))
    return postprocess(res.results)


# revision 14
# speedup vs baseline: 70.5727x; 1.0087x over previous
"""ChildSum tree RNN over a batch of complete binary trees — Trainium2 Bass kernel.

Strategy (data-parallel over trees, 8 cores x 128 trees):
  - States are kept *transposed* in SBUF ([feature, tree-node-column]) so every
    level's matmul chains directly into the next with no on-device transposes.
  - Block-local "deal" column order: within each block, the two siblings of a
    pair sit at offsets (p, p+HB), so the sibling-mean is a tensor_add of two
    contiguous bf16 runs (DVE 2x mode). The next level's matmul un-deals via a
    stride-2 rhs access pattern (free for the PE).
  - Level 0 computes both W_and/W_or transforms of the fp32 leaves (float32r
    matmuls) and selects per column with one copy_predicated on PSUM; the
    select commutes with the monotone tanh.
  - Levels 1-6 instead pre-scale the sibling-sums by host-baked bf16 masks
    am=0.5*(1-op), bm=0.5*op and accumulate W_and@s_and + W_or@s_or in a
    single PSUM tensor — no select pass, half the PSUM, 2048-wide tanh.
  - The mean's x0.5 lives in those masks / the halved level-0 handling is
    folded into pre-halved... level 0 mean feeds masked muls directly; the
    root mean's 0.5 is applied on the host.
"""

import sys

for _p in ("/opt/trn_rl_repo",):
    if _p not in sys.path:
        sys.path.insert(0, _p)

import numpy as np
import ml_dtypes

import concourse.bacc as bacc
import concourse.mybir as mybir
import concourse.tile as tile
from concourse import bass_utils

N_CORES = 8
B, L, M = 1024, 128, 256
BC = B // N_CORES          # trees per core
R0 = BC * L                # level-0 child columns per core (16384)
DEPTH = 7
LEVEL_R = [R0 >> l for l in range(DEPTH)]      # child columns per level
LEVEL_N = [64 >> l for l in range(DEPTH)]      # parents per tree per level
LEVEL_OFF = [0, 64, 96, 112, 120, 124, 126]    # offsets into ops[:, :]
BLK0 = 512                                     # level-0 child cols per block
BLK1 = 1024                                    # level>=1 child cols per block
MSK_SEG = [R0 >> (l + 1) for l in range(DEPTH - 1)]   # am/bm segment sizes
MSK_TOT = sum(MSK_SEG)                         # 16128

F32R = mybir.dt.float32r
F16 = mybir.dt.float16


def _body(nc, xT, wa, wo, wa5, wo5, mk, ma, mb, outT, tc):
    f32 = mybir.dt.float32
    u8 = mybir.dt.uint8

    with (
        tc.tile_pool(name="wpool", bufs=1) as wpool,
        tc.tile_pool(name="spool", bufs=1) as spool,
        tc.tile_pool(name="xpool", bufs=4) as xpool,
        tc.tile_pool(name="vpool", bufs=3) as vpool,
        tc.tile_pool(name="mpool", bufs=3) as mpool,
    ):
        # Stationary weights [contract-chunk m (128 part), out-feature k (256)]
        wt = {}
        for nm, dram, dt_ in (("wa", wa, F32R), ("wo", wo, F32R),
                              ("wa5", wa5, F16), ("wo5", wo5, F16)):
            for C in range(2):
                t = wpool.tile([128, 256], dt_, name=f"{nm}{C}", tag=f"{nm}{C}")
                nc.sync.dma_start(out=t, in_=dram[C * 128:(C + 1) * 128, :])
                wt[(nm, C)] = t

        # ---------- level 0: select design (fp32r x, copy_pred on PSUM) -----
        P0 = R0 // 2
        s_and = spool.tile([128, R0], F16, tag="sa_e", name="sand0")
        s_or = spool.tile([128, R0], F16, tag="so_e", name="sor0")
        with tc.tile_pool(name="ppool0", bufs=2, space="PSUM") as ppool0:
            HB = BLK0 // 2
            for j in range(R0 // BLK0):
                rhs = []
                for C in range(2):
                    xt = xpool.tile([128, BLK0], F32R, tag=f"x{C}",
                                    name=f"x{C}_{j}")
                    nc.sync.dma_start(
                        out=xt,
                        in_=xT[C * 128:(C + 1) * 128,
                               j * BLK0:(j + 1) * BLK0])
                    rhs.append(xt)

                ta = ppool0.tile([128, 2 * BLK0], f32, tag="ta", name=f"ta{j}")
                to_ = ppool0.tile([128, 2 * BLK0], f32, tag="to", name=f"to{j}")
                for psum_t, wnm in ((ta, "wa"), (to_, "wo")):
                    for Mo in range(2):
                        for C in range(2):
                            nc.tensor.matmul(
                                psum_t[:, Mo * BLK0:(Mo + 1) * BLK0],
                                wt[(wnm, C)][:, Mo * 128:(Mo + 1) * 128],
                                rhs[C],
                                start=(C == 0), stop=(C == 1))

                mt = mpool.tile([128, BLK0], u8, tag="mk", name=f"mk{j}")
                nc.sync.dma_start(
                    out=mt, in_=mk[:, j * BLK0:(j + 1) * BLK0])
                for Mo in range(2):
                    nc.vector.copy_predicated(
                        ta[:, Mo * BLK0:(Mo + 1) * BLK0], mt,
                        to_[:, Mo * BLK0:(Mo + 1) * BLK0])

                v = vpool.tile([128, 2 * BLK1], F16, tag="v", name=f"v{j}")
                nc.scalar.activation(v[:, 0:2 * BLK0], ta,
                                     mybir.ActivationFunctionType.Tanh)

                # sibling sum: halves of each (h, t, HB) block are contiguous
                v4 = v[:, 0:2 * BLK0].rearrange("p (h t q) -> p h t q",
                                                h=2, t=2)
                ss = vpool.tile([128, BLK1], F16, tag="ss", name=f"ss{j}")
                ss3 = ss[:, 0:2 * HB].rearrange("p (h q) -> p h q", h=2)
                nc.vector.tensor_add(ss3, v4[:, :, 0, :], v4[:, :, 1, :])

                # pre-scale for level 1: s_and = s*am, s_or = s*bm
                amt = mpool.tile([128, HB], F16, tag="am", name=f"am{j}")
                bmt = mpool.tile([128, HB], F16, tag="bm", name=f"bm{j}")
                nc.sync.dma_start(out=amt, in_=ma[:, j * HB:(j + 1) * HB])
                nc.sync.dma_start(out=bmt, in_=mb[:, j * HB:(j + 1) * HB])
                sa3 = s_and.rearrange("p (h q) -> p h q", h=2)
                so3 = s_or.rearrange("p (h q) -> p h q", h=2)
                am3 = amt.unsqueeze(1).broadcast_to([128, 2, HB])
                bm3 = bmt.unsqueeze(1).broadcast_to([128, 2, HB])
                nc.vector.tensor_mul(sa3[:, :, j * HB:(j + 1) * HB], ss3, am3)
                nc.vector.tensor_mul(so3[:, :, j * HB:(j + 1) * HB], ss3, bm3)

        # ---------- levels 1..6: premasked accumulation, single PSUM --------
        moff = MSK_SEG[0]
        sa_prev, so_prev = s_and, s_or
        with tc.tile_pool(name="ppool1", bufs=2, space="PSUM") as ppool1:
            for lvl in range(1, DEPTH):
                R = LEVEL_R[lvl]
                P = R // 2
                W = min(BLK1, R)
                HB = W // 2
                last = (lvl == DEPTH - 1)
                etag = "e" if lvl % 2 == 0 else "o"
                if not last:
                    s_and = spool.tile([128, R], F16, tag=f"sa_{etag}",
                                       name=f"sand{lvl}")
                    s_or = spool.tile([128, R], F16, tag=f"so_{etag}",
                                      name=f"sor{lvl}")
                else:
                    s_fin = spool.tile([128, 2 * BC], f32, tag="sfin",
                                       name="sfin")

                # rhs views: h-half C, sibling t, stride-2 over pairs
                sa_v = sa_prev.rearrange("p (h g t) -> p h t g", h=2, t=2)
                so_v = so_prev.rearrange("p (h g t) -> p h t g", h=2, t=2)

                for j in range(R // W):
                    T = ppool1.tile([128, 2 * W], f32, tag="tsel",
                                    name=f"T{lvl}_{j}")
                    nmm = HB // 512 if HB >= 512 else 1
                    NS = min(HB, 512)
                    for Mo in range(2):
                        for t in range(2):
                            for n in range(nmm):
                                g0 = j * HB + n * NS
                                out_ap = T[:, Mo * W + t * HB + n * NS:
                                           Mo * W + t * HB + n * NS + NS]
                                first = True
                                for sv, wnm in ((sa_v, "wa5"), (so_v, "wo5")):
                                    for C in range(2):
                                        nc.tensor.matmul(
                                            out_ap,
                                            wt[(wnm, C)][:, Mo * 128:
                                                         (Mo + 1) * 128],
                                            sv[:, C, t, g0:g0 + NS],
                                            start=first,
                                            stop=(wnm == "wo5" and C == 1))
                                        first = False

                    v = vpool.tile([128, 2 * BLK1], F16, tag="v",
                                   name=f"v{lvl}_{j}")
                    nc.scalar.activation(v[:, 0:2 * W], T,
                                         mybir.ActivationFunctionType.Tanh)

                    v4 = v[:, 0:2 * W].rearrange("p (h t q) -> p h t q",
                                                 h=2, t=2)
                    if not last:
                        ss = vpool.tile([128, BLK1], F16, tag="ss",
                                        name=f"ss{lvl}_{j}")
                        ss3 = ss[:, 0:2 * HB].rearrange("p (h q) -> p h q",
                                                        h=2)
                        nc.vector.tensor_add(ss3, v4[:, :, 0, :],
                                             v4[:, :, 1, :])
                        amt = mpool.tile([128, BLK1 // 2], F16, tag="am1",
                                         name=f"am{lvl}_{j}")
                        bmt = mpool.tile([128, BLK1 // 2], F16, tag="bm1",
                                         name=f"bm{lvl}_{j}")
                        nc.sync.dma_start(
                            out=amt[:, 0:HB],
                            in_=ma[:, moff + j * HB:moff + (j + 1) * HB])
                        nc.sync.dma_start(
                            out=bmt[:, 0:HB],
                            in_=mb[:, moff + j * HB:moff + (j + 1) * HB])
                        sa3 = s_and.rearrange("p (h q) -> p h q", h=2)
                        so3 = s_or.rearrange("p (h q) -> p h q", h=2)
                        am3 = amt[:, 0:HB].unsqueeze(1).broadcast_to(
                            [128, 2, HB])
                        bm3 = bmt[:, 0:HB].unsqueeze(1).broadcast_to(
                            [128, 2, HB])
                        nc.vector.tensor_mul(
                            sa3[:, :, j * HB:(j + 1) * HB], ss3, am3)
                        nc.vector.tensor_mul(
                            so3[:, :, j * HB:(j + 1) * HB], ss3, bm3)
                    else:
                        # root: fp32 sum, x0.5 applied on host
                        s3 = s_fin.rearrange("p (h q) -> p h q", h=2)
                        nc.vector.tensor_add(s3, v4[:, :, 0, :],
                                             v4[:, :, 1, :])

                if not last:
                    moff += MSK_SEG[lvl]
                    sa_prev, so_prev = s_and, s_or

        nc.sync.dma_start(out=outT, in_=s_fin)


_NC_CACHE = {}


def _get_nc(reps=1):
    key = ("nc", reps)
    if key not in _NC_CACHE:
        f32 = mybir.dt.float32
        u8 = mybir.dt.uint8
        nc = bacc.Bacc("TRN2", target_bir_lowering=False, debug=False)
        xT = nc.dram_tensor("xT", [M, R0], F32R, kind="ExternalInput").ap()
        wa = nc.dram_tensor("wa", [M, M], F32R, kind="ExternalInput").ap()
        wo = nc.dram_tensor("wo", [M, M], F32R, kind="ExternalInput").ap()
        wa5 = nc.dram_tensor("wa5", [M, M], F16, kind="ExternalInput").ap()
        wo5 = nc.dram_tensor("wo5", [M, M], F16, kind="ExternalInput").ap()
        mk = nc.dram_tensor("mk", [128, R0], u8, kind="ExternalInput").ap()
        ma = nc.dram_tensor("ma", [128, MSK_TOT], F16,
                            kind="ExternalInput").ap()
        mb = nc.dram_tensor("mb", [128, MSK_TOT], F16,
                            kind="ExternalInput").ap()
        outT = nc.dram_tensor("outT", [128, 2 * BC], f32,
                              kind="ExternalOutput").ap()
        with tile.TileContext(nc) as tc:
            for _ in range(reps):
                _body(nc, xT, wa, wo, wa5, wo5, mk, ma, mb, outT, tc)
        nc.compile()
        _NC_CACHE[key] = nc
    return _NC_CACHE[key]


def _deal_index():
    """pos -> flat leaf index (b*L + leaf) for the level-0 column order."""
    p = np.arange(R0)
    blk = p >> 9
    t = (p >> 8) & 1
    loc = p & 255
    g = blk * 256 + loc
    b = g >> 6
    i = g & 63
    return b * L + 2 * i + t, b, i


_DEAL = _deal_index()


def make_in_maps(inputs, ops, W_and, W_or):
    bf = np.float16
    x = np.asarray(inputs, dtype=np.float32)
    opsA = np.asarray(ops)
    waT = np.ascontiguousarray(np.asarray(W_and, dtype=np.float32).T)
    woT = np.ascontiguousarray(np.asarray(W_or, dtype=np.float32).T)
    # levels 1-6 weights at full scale (the mean's 0.5 lives in am/bm)
    wa5 = np.ascontiguousarray(waT.astype(bf))
    wo5 = np.ascontiguousarray(woT.astype(bf))
    leaf_idx, db, di = _DEAL
    in_maps = []
    for c in range(N_CORES):
        xc_flat = x[c * BC:(c + 1) * BC].reshape(BC * L, M)
        xc = np.ascontiguousarray(xc_flat[leaf_idx, :].T)
        opc = opsA[c * BC:(c + 1) * BC]
        # level-0 select mask in deal order: op of parent (b, i)
        mk0 = np.broadcast_to(
            opc[db, di].astype(np.uint8).reshape(1, R0), (128, R0))
        # premask rows (b-major child-res of levels 1..6)
        am_rows, bm_rows = [], []
        for lvl in range(1, DEPTH):
            n = LEVEL_N[lvl]
            off = LEVEL_OFF[lvl]
            row = np.repeat(opc[:, off:off + n], 2, axis=1).reshape(1, -1)
            am_rows.append(0.5 * (1.0 - row))
            bm_rows.append(0.5 * row)
        am = np.broadcast_to(np.concatenate(am_rows, 1).astype(bf),
                             (128, MSK_TOT))
        bm = np.broadcast_to(np.concatenate(bm_rows, 1).astype(bf),
                             (128, MSK_TOT))
        in_maps.append({
            "xT": xc, "wa": waT, "wo": woT, "wa5": wa5, "wo5": wo5,
            "mk": np.ascontiguousarray(mk0),
            "ma": np.ascontiguousarray(am),
            "mb": np.ascontiguousarray(bm),
        })
    return in_maps


def postprocess(results):
    outs = []
    for c in range(N_CORES):
        r = np.asarray(results[c]["outT"]).reshape(128, 2, BC)
        outs.append(0.5 * np.transpose(r, (2, 1, 0)).reshape(BC, M))
    return np.concatenate(outs, axis=0).astype(np.float32)


def kernel(inputs, ops, W_and, W_or):
    nc = _get_nc()
    in_maps = make_in_maps(inputs, ops, W_and, W_or)
    res = bass_utils.run_bass_kernel_spmd(nc, in_maps, list(range(N_CORES)))
    return postprocess(res.results)


# revision 15
# speedup vs baseline: 98.4521x; 1.3950x over previous
"""ChildSum tree RNN over a batch of complete binary trees — Trainium2 Bass kernel.

Strategy (data-parallel over trees, 8 cores x 128 trees):
  - States are kept *transposed* in SBUF ([feature, tree-node-column]) so every
    level's matmul chains directly into the next with no on-device transposes.
  - Block-local "deal" column order: within each block the two siblings of a
    pair sit at offsets (p, p+HB), so the sibling-mean is a tensor_add of two
    contiguous fp16 runs (DVE 2x mode). The next level's matmul un-deals via a
    stride-2 rhs access pattern (free for the PE).
  - Level 0 computes both W_and/W_or transforms of the fp32 leaves (float32r
    matmuls, full-rate PE) and selects per column with copy_predicated on
    PSUM; the select commutes with the monotone tanh.
  - Levels 1-6 instead pre-scale the sibling-sums (premasking): s_or = ss*bm
    with bm = 0.5*op (exact in fp16), s_and = 0.5*ss - s_or via
    scalar_tensor_tensor, then accumulate W_and@s_and + W_or@s_or in a single
    PSUM tensor — no select pass, half the PSUM, 2048-wide tanh.
  - DMA instruction count is kept minimal (fixed ~0.6us sequencer cost per
    dma_start): one up-front op-mask load, one bm load per level, 1024-column
    input loads; mask/weight DMAs issue from the idle GPSIMD sequencer.
  - The root mean's x0.5 is applied on the host.
"""

import sys

for _p in ("/opt/trn_rl_repo",):
    if _p not in sys.path:
        sys.path.insert(0, _p)

import numpy as np

import concourse.bacc as bacc
import concourse.mybir as mybir
import concourse.tile as tile
from concourse import bass_utils

N_CORES = 8
B, L, M = 1024, 128, 256
BC = B // N_CORES          # trees per core
R0 = BC * L                # level-0 child columns per core (16384)
DEPTH = 7
LEVEL_R = [R0 >> l for l in range(DEPTH)]      # child columns per level
LEVEL_N = [64 >> l for l in range(DEPTH)]      # parents per tree per level
LEVEL_OFF = [0, 64, 96, 112, 120, 124, 126]    # offsets into ops[:, :]
BLK0 = 512                                     # level-0 child cols per block
BLK1 = 1024                                    # level>=1 child cols per block
XSB = 1024                                     # level-0 x cols per DMA
MSK_SEG = [R0 >> (l + 1) for l in range(DEPTH - 1)]   # bm segment sizes
MSK_TOT = sum(MSK_SEG)                         # 16128

F32R = mybir.dt.float32r
F16 = mybir.dt.float16


def _body(nc, xT, wa, wo, wa5, wo5, mk, mb, outT, tc):
    f32 = mybir.dt.float32
    u8 = mybir.dt.uint8
    Alu = mybir.AluOpType

    with (
        tc.tile_pool(name="wpool", bufs=1) as wpool,
        tc.tile_pool(name="spool", bufs=1) as spool,
        tc.tile_pool(name="xpool", bufs=3) as xpool,
        tc.tile_pool(name="vpool", bufs=3) as vpool,
        tc.tile_pool(name="mpool", bufs=1) as mpool,
    ):
        # Stationary weights [contract-chunk m (128 part), out-feature k (256)]
        wt = {}
        for nm, dram, dt_ in (("wa", wa, F32R), ("wo", wo, F32R),
                              ("wa5", wa5, F16), ("wo5", wo5, F16)):
            for C in range(2):
                t = wpool.tile([128, 256], dt_, name=f"{nm}{C}", tag=f"{nm}{C}")
                nc.gpsimd.dma_start(out=t, in_=dram[C * 128:(C + 1) * 128, :])
                wt[(nm, C)] = t

        # one up-front load of the level-0 select mask (child-res, deal order)
        mk_t = mpool.tile([128, R0], u8, tag="mk", name="mk_t")
        nc.gpsimd.dma_start(out=mk_t, in_=mk)

        def epilogue(ss3, bm_t, s_and, s_or, j, HB):
            """premask: s_or = ss*bm, s_and = 0.5*ss - s_or (exact in fp16)"""
            sa3 = s_and.rearrange("p (h q) -> p h q", h=2)
            so3 = s_or.rearrange("p (h q) -> p h q", h=2)
            bm3 = (bm_t[:, j * HB:(j + 1) * HB]
                   .unsqueeze(1).broadcast_to([128, 2, HB]))
            osl = slice(j * HB, (j + 1) * HB)
            nc.vector.tensor_mul(so3[:, :, osl], ss3, bm3)
            nc.vector.scalar_tensor_tensor(
                sa3[:, :, osl], ss3, 0.5, so3[:, :, osl],
                Alu.mult, Alu.subtract)

        # ---------- level 0: select design (fp32r x, copy_pred on PSUM) -----
        s_and = spool.tile([128, R0], F16, tag="sa_e", name="sand0")
        s_or = spool.tile([128, R0], F16, tag="so_e", name="sor0")
        bm_t = mpool.tile([128, MSK_SEG[0]], F16, tag="bm", name="bm0")
        nc.gpsimd.dma_start(out=bm_t, in_=mb[:, 0:MSK_SEG[0]])
        with tc.tile_pool(name="ppool0", bufs=2, space="PSUM") as ppool0:
            HB = BLK0 // 2
            for sb in range(R0 // XSB):
                xts = []
                for C in range(2):
                    xt = xpool.tile([128, XSB], F32R, tag=f"x{C}",
                                    name=f"x{C}_{sb}")
                    nc.sync.dma_start(
                        out=xt,
                        in_=xT[C * 128:(C + 1) * 128, sb * XSB:(sb + 1) * XSB])
                    xts.append(xt)
                for ib in range(XSB // BLK0):
                    j = sb * (XSB // BLK0) + ib
                    rhs = [xts[C][:, ib * BLK0:(ib + 1) * BLK0]
                           for C in range(2)]

                    ta = ppool0.tile([128, 2 * BLK0], f32, tag="ta",
                                     name=f"ta{j}")
                    to_ = ppool0.tile([128, 2 * BLK0], f32, tag="to",
                                      name=f"to{j}")
                    for psum_t, wnm in ((ta, "wa"), (to_, "wo")):
                        for Mo in range(2):
                            for C in range(2):
                                nc.tensor.matmul(
                                    psum_t[:, Mo * BLK0:(Mo + 1) * BLK0],
                                    wt[(wnm, C)][:, Mo * 128:(Mo + 1) * 128],
                                    rhs[C],
                                    start=(C == 0), stop=(C == 1))

                    for Mo in range(2):
                        nc.vector.copy_predicated(
                            ta[:, Mo * BLK0:(Mo + 1) * BLK0],
                            mk_t[:, j * BLK0:(j + 1) * BLK0],
                            to_[:, Mo * BLK0:(Mo + 1) * BLK0])

                    v = vpool.tile([128, 2 * BLK1], F16, tag="v",
                                   name=f"v{j}")
                    nc.scalar.activation(v[:, 0:2 * BLK0], ta,
                                         mybir.ActivationFunctionType.Tanh)

                    v4 = v[:, 0:2 * BLK0].rearrange(
                        "p (h t q) -> p h t q", h=2, t=2)
                    ss = vpool.tile([128, BLK1], F16, tag="ss",
                                    name=f"ss{j}")
                    ss3 = ss[:, 0:2 * HB].rearrange("p (h q) -> p h q", h=2)
                    nc.vector.tensor_add(ss3, v4[:, :, 0, :], v4[:, :, 1, :])
                    epilogue(ss3, bm_t, s_and, s_or, j, HB)

        # ---------- levels 1..6: premasked accumulation, single PSUM --------
        moff = MSK_SEG[0]
        sa_prev, so_prev = s_and, s_or
        with tc.tile_pool(name="ppool1", bufs=2, space="PSUM") as ppool1:
            for lvl in range(1, DEPTH):
                R = LEVEL_R[lvl]
                W = min(BLK1, R)
                HB = W // 2
                last = (lvl == DEPTH - 1)
                etag = "e" if lvl % 2 == 0 else "o"
                if not last:
                    s_and = spool.tile([128, R], F16, tag=f"sa_{etag}",
                                       name=f"sand{lvl}")
                    s_or = spool.tile([128, R], F16, tag=f"so_{etag}",
                                      name=f"sor{lvl}")
                    bm_t = mpool.tile([128, MSK_SEG[0]], F16, tag="bm",
                                      name=f"bm{lvl}")
                    nc.gpsimd.dma_start(
                        out=bm_t[:, 0:MSK_SEG[lvl]],
                        in_=mb[:, moff:moff + MSK_SEG[lvl]])
                else:
                    s_fin = spool.tile([128, 2 * BC], f32, tag="sfin",
                                       name="sfin")

                # rhs views: h-half C, sibling t, stride-2 over pairs
                sa_v = sa_prev.rearrange("p (h g t) -> p h t g", h=2, t=2)
                so_v = so_prev.rearrange("p (h g t) -> p h t g", h=2, t=2)

                for j in range(R // W):
                    T = ppool1.tile([128, 2 * W], f32, tag="tsel",
                                    name=f"T{lvl}_{j}")
                    NS = min(HB, 512)
                    for Mo in range(2):
                        for t in range(2):
                            g0 = j * HB
                            out_ap = T[:, Mo * W + t * HB:
                                       Mo * W + t * HB + NS]
                            first = True
                            for sv, wnm in ((sa_v, "wa5"), (so_v, "wo5")):
                                for C in range(2):
                                    nc.tensor.matmul(
                                        out_ap,
                                        wt[(wnm, C)][:, Mo * 128:
                                                     (Mo + 1) * 128],
                                        sv[:, C, t, g0:g0 + NS],
                                        start=first,
                                        stop=(wnm == "wo5" and C == 1))
                                    first = False

                    v = vpool.tile([128, 2 * BLK1], F16, tag="v",
                                   name=f"v{lvl}_{j}")
                    nc.scalar.activation(v[:, 0:2 * W], T,
                                         mybir.ActivationFunctionType.Tanh)

                    v4 = v[:, 0:2 * W].rearrange("p (h t q) -> p h t q",
                                                 h=2, t=2)
                    if not last:
                        ss = vpool.tile([128, BLK1], F16, tag="ss",
                                        name=f"ss{lvl}_{j}")
                        ss3 = ss[:, 0:2 * HB].rearrange(
                            "p (h q) -> p h q", h=2)
                        nc.vector.tensor_add(ss3, v4[:, :, 0, :],
                                             v4[:, :, 1, :])
                        epilogue(ss3, bm_t, s_and, s_or, j, HB)
                    else:
                        # root: fp32 sum, x0.5 applied on host
                        s3 = s_fin.rearrange("p (h q) -> p h q", h=2)
                        nc.vector.tensor_add(s3, v4[:, :, 0, :],
                                             v4[:, :, 1, :])

                if not last:
                    moff += MSK_SEG[lvl]
                    sa_prev, so_prev = s_and, s_or

        nc.sync.dma_start(out=outT, in_=s_fin)


_NC_CACHE = {}


def _get_nc(reps=1):
    key = ("nc", reps)
    if key not in _NC_CACHE:
        f32 = mybir.dt.float32
        u8 = mybir.dt.uint8
        nc = bacc.Bacc("TRN2", target_bir_lowering=False, debug=False)
        xT = nc.dram_tensor("xT", [M, R0], F32R, kind="ExternalInput").ap()
        wa = nc.dram_tensor("wa", [M, M], F32R, kind="ExternalInput").ap()
        wo = nc.dram_tensor("wo", [M, M], F32R, kind="ExternalInput").ap()
        wa5 = nc.dram_tensor("wa5", [M, M], F16, kind="ExternalInput").ap()
        wo5 = nc.dram_tensor("wo5", [M, M], F16, kind="ExternalInput").ap()
        mk = nc.dram_tensor("mk", [128, R0], u8, kind="ExternalInput").ap()
        mb = nc.dram_tensor("mb", [128, MSK_TOT], F16,
                            kind="ExternalInput").ap()
        outT = nc.dram_tensor("outT", [128, 2 * BC], f32,
                              kind="ExternalOutput").ap()
        with tile.TileContext(nc) as tc:
            for _ in range(reps):
                _body(nc, xT, wa, wo, wa5, wo5, mk, mb, outT, tc)
        nc.compile()
        _NC_CACHE[key] = nc
    return _NC_CACHE[key]


def _deal_index():
    """pos -> flat leaf index (b*L + leaf) for the level-0 column order."""
    p = np.arange(R0)
    blk = p >> 9
    t = (p >> 8) & 1
    loc = p & 255
    g = blk * 256 + loc
    b = g >> 6
    i = g & 63
    return b * L + 2 * i + t, b, i


_DEAL = _deal_index()


def make_in_maps(inputs, ops, W_and, W_or):
    f16 = np.float16
    x = np.asarray(inputs, dtype=np.float32)
    opsA = np.asarray(ops)
    waT = np.ascontiguousarray(np.asarray(W_and, dtype=np.float32).T)
    woT = np.ascontiguousarray(np.asarray(W_or, dtype=np.float32).T)
    # levels 1-6 weights at full scale (the mean's 0.5 lives in bm / the
    # derived 0.5-ss term)
    wa5 = np.ascontiguousarray(waT.astype(f16))
    wo5 = np.ascontiguousarray(woT.astype(f16))
    leaf_idx, db, di = _DEAL
    in_maps = []
    for c in range(N_CORES):
        xc_flat = x[c * BC:(c + 1) * BC].reshape(BC * L, M)
        xc = np.ascontiguousarray(xc_flat[leaf_idx, :].T)
        opc = opsA[c * BC:(c + 1) * BC]
        # level-0 select mask in deal order: op of parent (b, i)
        mk0 = np.broadcast_to(
            opc[db, di].astype(np.uint8).reshape(1, R0), (128, R0))
        # bm rows (b-major child-res of levels 1..6): 0.5*op
        bm_rows = []
        for lvl in range(1, DEPTH):
            n = LEVEL_N[lvl]
            off = LEVEL_OFF[lvl]
            row = np.repeat(opc[:, off:off + n], 2, axis=1).reshape(1, -1)
            bm_rows.append(0.5 * row)
        bm = np.broadcast_to(np.concatenate(bm_rows, 1).astype(f16),
                             (128, MSK_TOT))
        in_maps.append({
            "xT": xc, "wa": waT, "wo": woT, "wa5": wa5, "wo5": wo5,
            "mk": np.ascontiguousarray(mk0),
            "mb": np.ascontiguousarray(bm),
        })
    return in_maps


def postprocess(results):
    outs = []
    for c in range(N_CORES):
        r = np.asarray(results[c]["outT"]).reshape(128, 2, BC)
        outs.append(0.5 * np.transpose(r, (2, 1, 0)).reshape(BC, M))
    return np.concatenate(outs, axis=0).astype(np.float32)


def kernel(inputs, ops, W_and, W_or):
    nc = _get_nc()
    in_maps = make_in_maps(inputs, ops, W_and, W_or)
    res = bass_utils.run_bass_kernel_spmd(nc, in_maps, list(range(N_CORES)))
    return postprocess(res.results)
